# revision 1
# baseline (speedup 1.0000x reference)
"""Trainium2 Bass kernel: binarized (XNOR/ReActNet-style) ResNet BasicBlock.

Computes, for x:[64,64,56,56] f32 and small per-channel parameters:

    out = PReLU_a(BN(conv3x3(sign(x + b0), scale * sign(w))) + x + b1) + b2

Distribution: data-parallel over the batch dim, 8 images per NeuronCore on
8 cores.  Per core, images (i, i+4) share the SBUF partition dim: channels
of the first image on partitions 0-63, channels of the second on 64-127.

Math folding (host side, all tiny tensors):
  - binarized weights sign(w) are pre-scaled by A_m = mean|w|_m * gamma_m /
    sqrt(var_m + eps)  (the BN multiplier), so PSUM holds BN-scaled conv.
    Products are +-A_m exactly, accumulated in fp32 PSUM -> only error is
    bf16 rounding of A_m itself (~2^-9 relative).
  - C_m = beta - mean*inv + bias1 + bias2 is applied as the ScalarE
    activation bias while reading PSUM.
  - residual +x is accumulated into PSUM with an identity matmul (bf16 x).
  - PReLU + bias2: y = max(t, a*t + d), d = bias2*(1-a), valid for a<=1;
    one fused DVE scalar_tensor_tensor when d==0, general 3-op path else.

On-chip layout: activations live in zero-padded 58x58 bf16 planes so each
3x3 tap is one contiguous 464-element matmul rhs slice; x and y live in
unpadded planes so HBM DMAs are 64 descriptors x 12.5KB contiguous.
Conv runs as 9+1 small matmuls per 8-row slice on 2x2 PE quadrants
(tile_position from partition bases); even/odd slices use complementary
quadrant pairs so four matmul streams run concurrently.
"""

import sys

if "/opt/trn_rl_repo" not in sys.path:
    sys.path.insert(0, "/opt/trn_rl_repo")

import numpy as np

import concourse.bass as bass
import concourse.bacc as bacc
import concourse.mybir as mybir
from concourse.tile import TileContext
from concourse.bass_utils import run_bass_kernel_spmd

AF = mybir.ActivationFunctionType
ALU = mybir.AluOpType
DT = mybir.dt

B, C, H, W = 64, 64, 56, 56
NCORES = 8
BPC = B // NCORES          # images per core
NPAIR = BPC // 2           # image pairs per core
HP, WP = H + 2, W + 2      # zero-padded plane 58x58
IMG = HP * WP              # 3364 elements per padded plane
PLN = H * W                # 3136 elements per unpadded plane
RB = 8                     # output rows per slice
NSL = H // RB              # 7 slices per image
NT = RB * WP               # 464: matmul free size (contiguous in padded space)
NI = RB * W                # 448: interior (valid) elements per slice
BN_EPS = 1e-5

_NC_CACHE = {}


def _build(fast_prelu: bool, reps: int = 1, probe: str = ''):
    nc = bacc.Bacc("TRN2", target_bir_lowering=False, debug=False)
    x_ext = nc.declare_dram_parameter("x", [BPC, C, H, W], DT.float32, isOutput=False)
    w_ext = nc.declare_dram_parameter("wts", [128, 10 * 64], DT.bfloat16, isOutput=False)
    c_ext = nc.declare_dram_parameter("cst", [128, 8], DT.float32, isOutput=False)
    o_ext = nc.declare_dram_parameter("out", [BPC, C, H, W], DT.float32, isOutput=True)

    xg = x_ext.ap().rearrange("b c h w -> b c (h w)")
    og = o_ext.ap().rearrange("b c h w -> b c (h w)")

    with TileContext(nc) as tc:
        with tc.tile_pool(name="persist", bufs=1) as perst, \
             tc.tile_pool(name="work", bufs=4) as work, \
             tc.tile_pool(name="psum", bufs=4, space="PSUM") as ppool:

            wts = perst.tile([128, 10 * 64], DT.bfloat16)
            nc.sync.dma_start(out=wts, in_=w_ext.ap())
            cst = perst.tile([128, 8], DT.float32)
            nc.sync.dma_start(out=cst, in_=c_ext.ap())
            c_ap = cst[:, 0:1]    # beta - mean*inv + bias1 + bias2
            a_ap = cst[:, 1:2]    # PReLU alpha
            d_ap = cst[:, 2:3]    # bias2 * (1 - alpha)
            b0_ap = cst[:, 3:4]   # bias0

            xf = perst.tile([128, NPAIR * PLN], DT.float32)    # unpadded planes
            yb = perst.tile([128, NPAIR * PLN], DT.float32)    # unpadded output
            act = perst.tile([128, NPAIR * IMG], DT.bfloat16)  # padded sign planes

            # residual staging: padded bf16 rows, pad columns zeroed once
            xbs = []
            for i in range(2):
                xb = perst.tile([128, NT], DT.bfloat16, name=f"xb{i}")
                v = xb.rearrange("p (r c) -> p r c", c=WP)
                nc.vector.memset(v[:, :, 0:1], 0.0)
                nc.vector.memset(v[:, :, WP - 1:WP], 0.0)
                xbs.append(xb)

            # zero the padding ring of every act plane
            for pr in range(NPAIR):
                v = act[:, pr * IMG:(pr + 1) * IMG].rearrange(
                    "p (h w) -> p h w", w=WP)
                nc.vector.memset(v[:, 0:1, :], 0.0)
                nc.vector.memset(v[:, HP - 1:HP, :], 0.0)
                nc.vector.memset(v[:, 1:HP - 1, 0:1], 0.0)
                nc.vector.memset(v[:, 1:HP - 1, WP - 1:WP], 0.0)

            def emit_loads(pr):
                ub = pr * PLN
                ab = pr * IMG
                nc.sync.dma_start(out=xf[0:64, ub:ub + PLN], in_=xg[pr])
                nc.sync.dma_start(out=xf[64:128, ub:ub + PLN], in_=xg[pr + NPAIR])
                for rc in range(NSL):
                    h0 = rc * RB
                    src = xf[:, ub + h0 * W:ub + h0 * W + NI].rearrange(
                        "p (r c) -> p r c", c=W)
                    adst = act[:, ab:ab + IMG].rearrange(
                        "p (h w) -> p h w", w=WP)[:, 1 + h0:1 + h0 + RB, 1:1 + W]
                    nc.scalar.activation(out=adst, in_=src, func=AF.Sign,
                                         bias=b0_ap, scale=1.0)

            def emit_xb_copy(s):
                pr, rc = divmod(s, NSL)
                h0 = rc * RB
                xb = xbs[s % 2]
                xb_i = xb.rearrange("p (r c) -> p r c", c=WP)[:, :, 1:1 + W]
                u0 = pr * PLN + h0 * W
                src = xf[:, u0:u0 + NI].rearrange("p (r c) -> p r c", c=W)
                nc.vector.tensor_copy(out=xb_i, in_=src)
                return xb

            def mm_args(s, t):
                pr, rc = divmod(s, NSL)
                h0 = rc * RB
                odd = s % 2
                if t == 9:      # identity (residual) step
                    la = wts[0:64, 576:640]
                    lb = wts[64:128, 576:640]
                    xb = xbs[odd]
                    ra, rb, s0, s1 = xb[0:64], xb[64:128], 0, NT
                else:
                    dh, dw = t // 3 - 1, t % 3 - 1
                    off = pr * IMG + (h0 + dh + 1) * WP + dw
                    s0 = 1 if (rc == 0 and dh == -1 and dw == -1) else 0
                    s1 = NT - 1 if (rc == NSL - 1 and dh == 1 and dw == 1) else NT
                    la = wts[0:64, t * 64:(t + 1) * 64]
                    lb = wts[64:128, t * 64:(t + 1) * 64]
                    ra = act[0:64, off + s0:off + s1]
                    rb = act[64:128, off + s0:off + s1]
                return la, lb, ra, rb, s0, s1, odd

            def emit_mms(slice_group):
                # interleave matmuls of an even+odd slice pair so all four
                # PE quadrants stream concurrently (starts are pc-monotone;
                # disjoint tile_positions overlap)
                pss = {}
                for s in slice_group:
                    pss[s] = ppool.tile([128, NT], DT.float32, tag="ps",
                                        name=f"ps{s}")
                # center tap first: never range-trimmed, so start=True clears
                # the whole bank before the trimmed corner taps accumulate
                for t in (4, 0, 1, 2, 3, 5, 6, 7, 8, 9):
                    for s in slice_group:
                        la, lb, ra, rb, s0, s1, odd = mm_args(s, t)
                        ps = pss[s]
                        pa = ps[64:128, s0:s1] if odd else ps[0:64, s0:s1]
                        pb = ps[0:64, s0:s1] if odd else ps[64:128, s0:s1]
                        nc.tensor.matmul(pa, la, ra, start=(t == 4),
                                         stop=(t == 9), skip_group_check=True)
                        nc.tensor.matmul(pb, lb, rb, start=(t == 4),
                                         stop=(t == 9), skip_group_check=True)
                return pss

            def emit_epilogue(s, ps):
                pr, rc = divmod(s, NSL)
                h0 = rc * RB
                u0 = pr * PLN + h0 * W
                tt = work.tile([128, NI], DT.float32, tag="tt")
                ps_i = ps.rearrange("p (r c) -> p r c", c=WP)[:, :, 1:1 + W]
                tt_v = tt.rearrange("p (r c) -> p r c", c=W)
                nc.scalar.activation(out=tt_v, in_=ps_i, func=AF.Identity,
                                     bias=c_ap, scale=1.0)
                yv = yb[:, u0:u0 + NI]
                if fast_prelu:
                    # y = max(t, a*t); valid since d == 0 and a <= 1
                    nc.vector.scalar_tensor_tensor(
                        out=yv, in0=tt, scalar=a_ap, in1=tt,
                        op0=ALU.mult, op1=ALU.max)
                else:
                    # y = max(t, 0) + d  +  a * min(t, 0); any a, d
                    vv = work.tile([128, NI], DT.float32, tag="vv")
                    nc.vector.tensor_scalar(vv, tt, 0.0, a_ap,
                                            op0=ALU.min, op1=ALU.mult)
                    nc.vector.tensor_scalar(tt, tt, 0.0, d_ap,
                                            op0=ALU.max, op1=ALU.add)
                    nc.vector.tensor_add(yv, tt, vv)

            def emit_store(pr):
                # output store, split by slice parity: odd-parity slices have
                # swapped halves (image B on partitions 0-63) from the crossed
                # PE quadrants, so route each parity's chunks separately
                ub = pr * PLN
                ia, ib = pr, pr + NPAIR
                ov_a = og[ia].rearrange("c (k n) -> c k n", n=NI)
                ov_b = og[ib].rearrange("c (k n) -> c k n", n=NI)
                yv_lo = yb[0:64, ub:ub + PLN].rearrange("c (k n) -> c k n", n=NI)
                yv_hi = yb[64:128, ub:ub + PLN].rearrange("c (k n) -> c k n", n=NI)
                pn = pr % 2          # rc parity whose layout is normal [A|B]
                psw = 1 - pn
                # stores go out on the Activation HWDGE queue so they never
                # head-of-line block the input loads on the SP queue
                nc.scalar.dma_start(out=ov_a[:, pn:NSL:2], in_=yv_lo[:, pn:NSL:2])
                nc.scalar.dma_start(out=ov_b[:, pn:NSL:2], in_=yv_hi[:, pn:NSL:2])
                nc.scalar.dma_start(out=ov_a[:, psw:NSL:2], in_=yv_hi[:, psw:NSL:2])
                nc.scalar.dma_start(out=ov_b[:, psw:NSL:2], in_=yv_lo[:, psw:NSL:2])

            def emit_compute_all():
                if probe == 'pe':
                    for s0 in range(0, NPAIR * NSL - 1, 2):
                        emit_mms([s0, s0 + 1])
                    return
                nsl_tot = NPAIR * NSL
                s = 0
                while s < nsl_tot:
                    group = [s] if s + 1 >= nsl_tot else [s, s + 1]
                    for g in group:
                        emit_xb_copy(g)
                    if probe == 'nope':
                        pss = {g: ppool.tile([128, NT], DT.float32, tag="ps",
                                             name=f"ps{g}") for g in group}
                        for g in group:
                            nc.vector.memset(pss[g][:, 0:1], 0.0)
                    else:
                        pss = emit_mms(group)
                    for g in group:
                        emit_epilogue(g, pss[g])
                    for g in group:
                        if (g + 1) % NSL == 0:
                            emit_store(g // NSL)
                    s += len(group)

            for _ in range(reps):
                if probe != 'pe':
                    # all x loads + signs up front: signs clear the ACT FIFO
                    # before the PSUM affines start queueing behind them
                    for pr in range(NPAIR):
                        emit_loads(pr)
                emit_compute_all()

    nc.compile()
    return nc


def _get_nc(fast_prelu: bool, reps: int = 1, probe: str = ''):
    key = (fast_prelu, reps, probe)
    if key not in _NC_CACHE:
        _NC_CACHE[key] = _build(fast_prelu, reps, probe)
    return _NC_CACHE[key]


def _prepare(x, bias0, w, gamma, beta, run_mean, run_var, bias1, alpha, bias2):
    bf16 = DT.np(DT.bfloat16)
    x = np.ascontiguousarray(np.asarray(x, np.float32))
    w = np.asarray(w, np.float32)
    sw = np.sign(w)                                   # [P, C, 3, 3]
    scale = np.abs(w).mean(axis=(1, 2, 3))            # [P]
    inv = np.asarray(gamma, np.float32) / np.sqrt(
        np.asarray(run_var, np.float32) + np.float32(BN_EPS))
    A = (scale * inv).astype(np.float32)
    b1 = np.asarray(bias1, np.float32).reshape(-1)
    b2 = np.asarray(bias2, np.float32).reshape(-1)
    al = np.asarray(alpha, np.float32).reshape(-1)
    b0 = np.asarray(bias0, np.float32).reshape(-1)
    Cc = (np.asarray(beta, np.float32) -
          np.asarray(run_mean, np.float32) * inv + b1 + b2).astype(np.float32)
    dd = (b2 * (1.0 - al)).astype(np.float32)

    wt = np.zeros((128, 640), np.float32)
    for t in range(9):
        blk = (sw[:, :, t // 3, t % 3] * A[:, None]).T      # [C, P]
        wt[0:64, t * 64:(t + 1) * 64] = blk
        wt[64:128, t * 64:(t + 1) * 64] = blk
    ident = np.eye(64, dtype=np.float32)
    wt[0:64, 576:640] = ident
    wt[64:128, 576:640] = ident
    wt_bf = np.ascontiguousarray(wt.astype(bf16))

    cst = np.zeros((128, 8), np.float32)
    for half in range(2):
        sl = slice(half * 64, half * 64 + 64)
        cst[sl, 0] = Cc
        cst[sl, 1] = al
        cst[sl, 2] = dd
        cst[sl, 3] = b0

    fast_prelu = bool(np.all(dd == 0.0) and np.all(al <= 1.0))
    in_maps = [
        {"x": np.ascontiguousarray(x[c * BPC:(c + 1) * BPC]),
         "wts": wt_bf, "cst": cst}
        for c in range(NCORES)
    ]
    return in_maps, fast_prelu


_RUNNER_CACHE = {}


def _make_runner(nc, n_cores=NCORES):
    """Build a reusable jitted executor for `nc` (one XLA trace, NEFF cached)."""
    import jax
    from jax.sharding import Mesh, PartitionSpec, NamedSharding
    from jax.experimental.shard_map import shard_map
    from concourse import bass2jax

    bass2jax.install_neuronx_cc_hook()
    partition_name = nc.partition_id_tensor.name if nc.partition_id_tensor else None
    in_names, out_names, out_avals, zero_outs = [], [], [], []
    for alloc in nc.m.functions[0].allocations:
        if not isinstance(alloc, mybir.MemoryLocationSet):
            continue
        name = alloc.memorylocations[0].name
        if alloc.kind == "ExternalInput":
            if name != partition_name:
                in_names.append(name)
        elif alloc.kind == "ExternalOutput":
            out_names.append(name)
            shape = tuple(alloc.tensor_shape)
            dtype = mybir.dt.np(alloc.dtype)
            out_avals.append(jax.core.ShapedArray(shape, dtype))
            zero_outs.append(np.zeros(shape, dtype))
    n_params = len(in_names)
    all_in = list(in_names) + out_names + ([partition_name] if partition_name else [])

    def _body(*args):
        operands = list(args)
        if partition_name is not None:
            operands.append(bass2jax.partition_id_tensor())
        outs = bass2jax._bass_exec_p.bind(
            *operands,
            out_avals=tuple(out_avals),
            in_names=tuple(all_in),
            out_names=tuple(out_names),
            lowering_input_output_aliases=(),
            sim_require_finite=True,
            sim_require_nnan=True,
            nc=nc,
        )
        return tuple(outs)

    devices = jax.devices()[:n_cores]
    mesh = Mesh(np.asarray(devices), ("core",))
    nin = n_params + len(out_names)
    f = jax.jit(shard_map(
        _body, mesh=mesh,
        in_specs=(PartitionSpec("core"),) * nin,
        out_specs=(PartitionSpec("core"),) * len(out_names),
        check_rep=False))
    sh = NamedSharding(mesh, PartitionSpec("core"))
    concat_zeros = [
        jax.device_put(np.zeros((n_cores * z.shape[0], *z.shape[1:]), z.dtype), sh)
        for z in zero_outs
    ]

    def run(in_maps):
        concat_in = [
            np.concatenate([np.asarray(in_maps[c][nm]) for c in range(n_cores)],
                           axis=0)
            for nm in in_names
        ]
        args = [jax.device_put(a, sh) for a in concat_in] + concat_zeros
        outs = f(*args)
        jax.block_until_ready(outs)
        oi = out_names.index("out")
        full = np.asarray(outs[oi])
        return full.reshape(n_cores, *out_avals[oi].shape)

    run.jit_fn = f
    run.sharding = sh
    run.in_names = in_names
    run.out_names = out_names
    run.zero_args = concat_zeros
    return run


def _get_runner(fast_prelu: bool, reps: int = 1, probe: str = ''):
    key = (fast_prelu, reps, probe)
    if key not in _RUNNER_CACHE:
        _RUNNER_CACHE[key] = _make_runner(_get_nc(fast_prelu, reps, probe))
    return _RUNNER_CACHE[key]


def _run(inputs: dict, trace: bool = False, reps: int = 1, **spmd_kwargs):
    """Legacy path through run_bass_kernel_spmd (used for debugging)."""
    in_maps, fast_prelu = _prepare(**inputs)
    nc = _get_nc(fast_prelu, reps)
    res = run_bass_kernel_spmd(nc, in_maps, list(range(NCORES)),
                               trace=trace, **spmd_kwargs)
    out = np.concatenate([res.results[c]["out"] for c in range(NCORES)], axis=0)
    return out, res


def kernel(**inputs) -> np.ndarray:
    in_maps, fast_prelu = _prepare(**inputs)
    runner = _get_runner(fast_prelu)
    per_core = runner(in_maps)
    return np.ascontiguousarray(per_core.reshape(B, C, H, W))



# revision 2
# speedup vs baseline: 2.3132x; 2.3132x over previous
"""Trainium2 Bass kernel: binarized (XNOR/ReActNet-style) ResNet BasicBlock.

Computes, for x:[64,64,56,56] f32 and small per-channel parameters:

    out = PReLU_a(BN(conv3x3(sign(x + b0), scale * sign(w))) + x + b1) + b2

Distribution: data-parallel over the batch dim, 8 images per NeuronCore on
8 cores.  Per core, images (i, i+4) share the SBUF partition dim: channels
of the first image on partitions 0-63, channels of the second on 64-127.

Design (v2):
  - HBM I/O in bf16 both ways (x is binarized on-chip and the residual path
    was already bf16; output rounding ~2^-9 rel).  Halves DMA time.
  - Binarization as u = (x >= -b0) in {0,1} on the DVE (is_ge, 4x mode)
    instead of ScalarE Sign.  Padding ring holds 0.5 so s = 2u-1 = 0 there;
    conv = sum 2*W*u - sum W, with the per-channel constant sum_W folded
    into the epilogue bias.  Weights are pre-scaled by 2*A_m (A = mean|w| *
    gamma/sqrt(var+eps)), so PSUM holds the BN-scaled conv.
  - Residual +x accumulated into PSUM with an identity matmul on bf16 x
    (also performs the odd-slice partition swap from the crossed quadrants).
  - Whole epilogue is ONE ScalarE op per slice: y = Prelu(ps + C; alpha)
    with per-partition bias C and slope alpha, reading PSUM, writing bf16.
  - Matmuls use interior-only strided rhs APs (8 rows x 56 of the padded
    58-wide planes), 448 columns, so no staging copies and no pad waste.
  - Act planes are right-pad-2 layout: data at cols 0-55, pads at cols
    56-57 (col -1 of row r aliases col 57 of row r-1), so DVE 4x alignment
    holds and every tap is a clean strided view.
  - Conv runs as 10 small matmuls per 8-row slice on 2x2 PE quadrants
    (tile_position from partition bases); even/odd slices use complementary
    quadrant pairs so four matmul streams run concurrently.
"""

import sys

if "/opt/trn_rl_repo" not in sys.path:
    sys.path.insert(0, "/opt/trn_rl_repo")

import numpy as np

import concourse.bass as bass
import concourse.bacc as bacc
import concourse.mybir as mybir
from concourse.tile import TileContext
from concourse.bass_utils import run_bass_kernel_spmd

AF = mybir.ActivationFunctionType
ALU = mybir.AluOpType
DT = mybir.dt

B, C, H, W = 64, 64, 56, 56
NCORES = 8
BPC = B // NCORES          # images per core
NPAIR = BPC // 2           # image pairs per core
WP = W + 2                 # padded row stride 58
HP = H + 2                 # padded plane rows 58
IMG = HP * WP              # 3364 elements per padded plane
PLN = H * W                # 3136 elements per unpadded plane
RB = 8                     # output rows per slice
NSL = H // RB              # 7 slices per image
NI = RB * W                # 448 interior elements per slice
GOFF = 2                   # guard elements before plane 0 (tap dw=-1 at row 0)
ACTSZ = GOFF + NPAIR * IMG + 2
PAD = 0.5                  # pad value: s = 2*0.5 - 1 = 0
BN_EPS = 1e-5

_NC_CACHE = {}


def _build(fast_prelu: bool, reps: int = 1, probe: str = ''):
    nc = bacc.Bacc("TRN2", target_bir_lowering=False, debug=False)
    x_ext = nc.declare_dram_parameter("x", [BPC, C, H, W], DT.bfloat16,
                                      isOutput=False)
    w_ext = nc.declare_dram_parameter("wts", [128, 10 * 64], DT.bfloat16,
                                      isOutput=False)
    c_ext = nc.declare_dram_parameter("cst", [128, 8], DT.float32,
                                      isOutput=False)
    o_ext = nc.declare_dram_parameter("out", [BPC, C, H, W], DT.bfloat16,
                                      isOutput=True)

    xg = x_ext.ap().rearrange("b c h w -> b c (h w)")
    og = o_ext.ap().rearrange("b c h w -> b c (h w)")

    with TileContext(nc) as tc:
        with tc.tile_pool(name="persist", bufs=1) as perst, \
             tc.tile_pool(name="work", bufs=4) as work, \
             tc.tile_pool(name="psum", bufs=4, space="PSUM") as ppool:

            wts = perst.tile([128, 10 * 64], DT.bfloat16)
            nc.sync.dma_start(out=wts, in_=w_ext.ap())
            cst = perst.tile([128, 8], DT.float32)
            nc.sync.dma_start(out=cst, in_=c_ext.ap())
            c_ap = cst[:, 0:1]    # beta - mean*inv + b1 + b2 - A*K
            a_ap = cst[:, 1:2]    # PReLU alpha
            d_ap = cst[:, 2:3]    # bias2 * (1 - alpha)
            nb0_ap = cst[:, 3:4]  # -bias0 (is_ge threshold)

            xf = perst.tile([128, NPAIR * PLN], DT.bfloat16)   # raw x planes
            yb = perst.tile([128, NPAIR * PLN], DT.bfloat16)   # output planes
            act = perst.tile([128, ACTSZ], DT.bfloat16)        # padded 0/1

            # pad ring = 0.5 everywhere (top/bottom rows, right-pad-2 cols,
            # and the guard elements at both ends)
            nc.vector.memset(act[:, 0:GOFF], PAD)
            nc.vector.memset(act[:, GOFF + NPAIR * IMG:], PAD)
            for pr in range(NPAIR):
                v = act[:, GOFF + pr * IMG:GOFF + (pr + 1) * IMG].rearrange(
                    "p (h w) -> p h w", w=WP)
                nc.vector.memset(v[:, 0:1, :], PAD)
                nc.vector.memset(v[:, HP - 1:HP, :], PAD)
                nc.vector.memset(v[:, 1:HP - 1, W:WP], PAD)

            def emit_load(pr):
                ub = pr * PLN
                nc.sync.dma_start(out=xf[0:64, ub:ub + PLN], in_=xg[pr])
                nc.sync.dma_start(out=xf[64:128, ub:ub + PLN],
                                  in_=xg[pr + NPAIR])

            def emit_sign(pr):
                ub = pr * PLN
                src = xf[:, ub:ub + PLN].rearrange("p (h w) -> p h w", w=W)
                dst = act[:, GOFF + pr * IMG:GOFF + (pr + 1) * IMG].rearrange(
                    "p (h w) -> p h w", w=WP)[:, 1:1 + H, 0:W]
                nc.vector.tensor_scalar(dst, src, nb0_ap, None, op0=ALU.is_ge)

            def mm_args(s, t):
                pr, rc = divmod(s, NSL)
                h0 = rc * RB
                if t == 9:      # identity (residual) step, bf16 raw x
                    u0 = pr * PLN + h0 * W
                    la = wts[0:64, 576:640]
                    lb = wts[64:128, 576:640]
                    ra = xf[0:64, u0:u0 + NI]
                    rb = xf[64:128, u0:u0 + NI]
                else:
                    dh, dw = t // 3 - 1, t % 3 - 1
                    X = GOFF + pr * IMG + (1 + h0 + dh) * WP + dw
                    va = act[0:64, X:X + RB * WP].rearrange(
                        "p (r c) -> p r c", c=WP)[:, :, 0:W]
                    vb = act[64:128, X:X + RB * WP].rearrange(
                        "p (r c) -> p r c", c=WP)[:, :, 0:W]
                    la = wts[0:64, t * 64:(t + 1) * 64]
                    lb = wts[64:128, t * 64:(t + 1) * 64]
                    ra, rb = va, vb
                return la, lb, ra, rb

            def emit_mms(slice_group):
                # interleave matmuls of an even+odd slice pair so all four
                # PE quadrants stream concurrently
                pss = {}
                for s in slice_group:
                    pss[s] = ppool.tile([128, NI], DT.float32, tag="ps",
                                        name=f"ps{s}")
                for t in range(10):
                    for s in slice_group:
                        la, lb, ra, rb = mm_args(s, t)
                        ps = pss[s]
                        odd = s % 2
                        pa = ps[64:128] if odd else ps[0:64]
                        pb = ps[0:64] if odd else ps[64:128]
                        nc.tensor.matmul(pa, la, ra, start=(t == 0),
                                         stop=(t == 9), skip_group_check=True)
                        nc.tensor.matmul(pb, lb, rb, start=(t == 0),
                                         stop=(t == 9), skip_group_check=True)
                return pss

            def emit_epilogue(s, ps):
                pr, rc = divmod(s, NSL)
                u0 = pr * PLN + rc * RB * W
                yv = yb[:, u0:u0 + NI]
                if fast_prelu:
                    # y = Prelu(ps + C; alpha): one ScalarE op, PSUM -> bf16
                    nc.scalar.activation(out=yv, in_=ps, func=AF.Prelu,
                                         bias=c_ap, scale=1.0, alpha=a_ap)
                else:
                    # y = max(t,0) + d + a*min(t,0), t = ps + C; any a, d
                    tt = work.tile([128, NI], DT.float32, tag="tt")
                    nc.scalar.activation(out=tt, in_=ps, func=AF.Identity,
                                         bias=c_ap, scale=1.0)
                    vv = work.tile([128, NI], DT.float32, tag="vv")
                    nc.vector.tensor_scalar(vv, tt, 0.0, a_ap,
                                            op0=ALU.min, op1=ALU.mult)
                    nc.vector.tensor_scalar(tt, tt, 0.0, d_ap,
                                            op0=ALU.max, op1=ALU.add)
                    nc.vector.tensor_add(yv, tt, vv)

            def emit_store(pr):
                # output store, split by slice parity: odd-parity slices have
                # swapped halves (image B on partitions 0-63) from the crossed
                # PE quadrants, so route each parity's chunks separately
                ub = pr * PLN
                ia, ib = pr, pr + NPAIR
                ov_a = og[ia].rearrange("c (k n) -> c k n", n=NI)
                ov_b = og[ib].rearrange("c (k n) -> c k n", n=NI)
                yv_lo = yb[0:64, ub:ub + PLN].rearrange("c (k n) -> c k n", n=NI)
                yv_hi = yb[64:128, ub:ub + PLN].rearrange("c (k n) -> c k n", n=NI)
                pn = pr % 2          # rc parity whose layout is normal [A|B]
                psw = 1 - pn
                # stores go out on the Activation HWDGE queue so they never
                # head-of-line block the input loads on the SP queue
                nc.scalar.dma_start(out=ov_a[:, pn:NSL:2], in_=yv_lo[:, pn:NSL:2])
                nc.scalar.dma_start(out=ov_b[:, pn:NSL:2], in_=yv_hi[:, pn:NSL:2])
                nc.scalar.dma_start(out=ov_a[:, psw:NSL:2], in_=yv_hi[:, psw:NSL:2])
                nc.scalar.dma_start(out=ov_b[:, psw:NSL:2], in_=yv_lo[:, psw:NSL:2])

            def emit_compute_all():
                if probe == 'pe':
                    for s0 in range(0, NPAIR * NSL - 1, 2):
                        emit_mms([s0, s0 + 1])
                    return
                nsl_tot = NPAIR * NSL
                s = 0
                while s < nsl_tot:
                    group = [s] if s + 1 >= nsl_tot else [s, s + 1]
                    if probe == 'nope':
                        pss = {g: ppool.tile([128, NI], DT.float32, tag="ps",
                                             name=f"ps{g}") for g in group}
                        for g in group:
                            nc.vector.memset(pss[g][:, 0:1], 0.0)
                    else:
                        pss = emit_mms(group)
                    for g in group:
                        emit_epilogue(g, pss[g])
                    for g in group:
                        if (g + 1) % NSL == 0:
                            emit_store(g // NSL)
                    s += len(group)

            for _ in range(reps):
                if probe != 'pe':
                    for pr in range(NPAIR):
                        emit_load(pr)
                    for pr in range(NPAIR):
                        emit_sign(pr)
                emit_compute_all()

    nc.compile()
    return nc


def _get_nc(fast_prelu: bool, reps: int = 1, probe: str = ''):
    key = (fast_prelu, reps, probe)
    if key not in _NC_CACHE:
        _NC_CACHE[key] = _build(fast_prelu, reps, probe)
    return _NC_CACHE[key]


def _prepare(x, bias0, w, gamma, beta, run_mean, run_var, bias1, alpha, bias2):
    bf16 = DT.np(DT.bfloat16)
    x = np.asarray(x, np.float32)
    w = np.asarray(w, np.float32)
    sw = np.sign(w)                                   # [P, C, 3, 3]
    scale = np.abs(w).mean(axis=(1, 2, 3))            # [P]
    inv = np.asarray(gamma, np.float32) / np.sqrt(
        np.asarray(run_var, np.float32) + np.float32(BN_EPS))
    A = (scale * inv).astype(np.float32)
    K = sw.sum(axis=(1, 2, 3)).astype(np.float32)     # sum of weight signs
    b1 = np.asarray(bias1, np.float32).reshape(-1)
    b2 = np.asarray(bias2, np.float32).reshape(-1)
    al = np.asarray(alpha, np.float32).reshape(-1)
    b0 = np.asarray(bias0, np.float32).reshape(-1)
    Cc = (np.asarray(beta, np.float32) -
          np.asarray(run_mean, np.float32) * inv + b1 + b2 - A * K
          ).astype(np.float32)
    dd = (b2 * (1.0 - al)).astype(np.float32)

    wt = np.zeros((128, 640), np.float32)
    for t in range(9):
        blk = (sw[:, :, t // 3, t % 3] * (2.0 * A)[:, None]).T    # [C, P]
        wt[0:64, t * 64:(t + 1) * 64] = blk
        wt[64:128, t * 64:(t + 1) * 64] = blk
    ident = np.eye(64, dtype=np.float32)
    wt[0:64, 576:640] = ident
    wt[64:128, 576:640] = ident
    wt_bf = np.ascontiguousarray(wt.astype(bf16))

    cst = np.zeros((128, 8), np.float32)
    for half in range(2):
        sl = slice(half * 64, half * 64 + 64)
        cst[sl, 0] = Cc
        cst[sl, 1] = al
        cst[sl, 2] = dd
        cst[sl, 3] = -b0

    fast_prelu = bool(np.all(dd == 0.0))
    x_bf = x.astype(bf16)
    in_maps = [
        {"x": np.ascontiguousarray(x_bf[c * BPC:(c + 1) * BPC]),
         "wts": wt_bf, "cst": cst}
        for c in range(NCORES)
    ]
    return in_maps, fast_prelu


_RUNNER_CACHE = {}


def _make_runner(nc, n_cores=NCORES):
    """Build a reusable jitted executor for `nc` (one XLA trace, NEFF cached)."""
    import jax
    from jax.sharding import Mesh, PartitionSpec, NamedSharding
    from jax.experimental.shard_map import shard_map
    from concourse import bass2jax

    bass2jax.install_neuronx_cc_hook()
    partition_name = nc.partition_id_tensor.name if nc.partition_id_tensor else None
    in_names, out_names, out_avals, zero_outs = [], [], [], []
    for alloc in nc.m.functions[0].allocations:
        if not isinstance(alloc, mybir.MemoryLocationSet):
            continue
        name = alloc.memorylocations[0].name
        if alloc.kind == "ExternalInput":
            if name != partition_name:
                in_names.append(name)
        elif alloc.kind == "ExternalOutput":
            out_names.append(name)
            shape = tuple(alloc.tensor_shape)
            dtype = mybir.dt.np(alloc.dtype)
            out_avals.append(jax.core.ShapedArray(shape, dtype))
            zero_outs.append(np.zeros(shape, dtype))
    n_params = len(in_names)
    all_in = list(in_names) + out_names + ([partition_name] if partition_name else [])

    def _body(*args):
        operands = list(args)
        if partition_name is not None:
            operands.append(bass2jax.partition_id_tensor())
        outs = bass2jax._bass_exec_p.bind(
            *operands,
            out_avals=tuple(out_avals),
            in_names=tuple(all_in),
            out_names=tuple(out_names),
            lowering_input_output_aliases=(),
            sim_require_finite=True,
            sim_require_nnan=True,
            nc=nc,
        )
        return tuple(outs)

    devices = jax.devices()[:n_cores]
    mesh = Mesh(np.asarray(devices), ("core",))
    nin = n_params + len(out_names)
    f = jax.jit(shard_map(
        _body, mesh=mesh,
        in_specs=(PartitionSpec("core"),) * nin,
        out_specs=(PartitionSpec("core"),) * len(out_names),
        check_rep=False))
    sh = NamedSharding(mesh, PartitionSpec("core"))
    concat_zeros = [
        jax.device_put(np.zeros((n_cores * z.shape[0], *z.shape[1:]), z.dtype), sh)
        for z in zero_outs
    ]

    def run(in_maps):
        concat_in = [
            np.concatenate([np.asarray(in_maps[c][nm]) for c in range(n_cores)],
                           axis=0)
            for nm in in_names
        ]
        args = [jax.device_put(a, sh) for a in concat_in] + concat_zeros
        outs = f(*args)
        jax.block_until_ready(outs)
        oi = out_names.index("out")
        full = np.asarray(outs[oi])
        return full.reshape(n_cores, *out_avals[oi].shape)

    run.jit_fn = f
    run.sharding = sh
    run.in_names = in_names
    run.out_names = out_names
    run.zero_args = concat_zeros
    return run


def _get_runner(fast_prelu: bool, reps: int = 1, probe: str = ''):
    key = (fast_prelu, reps, probe)
    if key not in _RUNNER_CACHE:
        _RUNNER_CACHE[key] = _make_runner(_get_nc(fast_prelu, reps, probe))
    return _RUNNER_CACHE[key]


def _run(inputs: dict, trace: bool = False, reps: int = 1, **spmd_kwargs):
    """Legacy path through run_bass_kernel_spmd (used for debugging)."""
    in_maps, fast_prelu = _prepare(**inputs)
    nc = _get_nc(fast_prelu, reps)
    res = run_bass_kernel_spmd(nc, in_maps, list(range(NCORES)),
                               trace=trace, **spmd_kwargs)
    out = np.concatenate([res.results[c]["out"] for c in range(NCORES)], axis=0)
    return out, res


def kernel(**inputs) -> np.ndarray:
    in_maps, fast_prelu = _prepare(**inputs)
    runner = _get_runner(fast_prelu)
    per_core = runner(in_maps)
    return np.ascontiguousarray(
        per_core.reshape(B, C, H, W).astype(np.float32))
